# revision 12
# baseline (speedup 1.0000x reference)
"""Trainium2 Bass kernel for the Centroid (segment_reduce) problem.

new_centroid = 0.3 * (segment_sum(embed, y) / counts) + 0.7 * centroid
  embed [32768, 1024] f32, y [32768] int (0..999), centroid [1000, 1024] f32

Strategy (8 NeuronCores, CLASS-parallel via host-side routing):
  - The host partitions the 1000 classes into 8 groups of <=128 classes,
    balanced by sample count (LPT + swap refinement; for the uniform
    label distribution every group lands at ~4096 of the 32768 samples).
  - Core i receives ONLY the embed rows whose label falls in its group
    (as fp8 e4m3 with a trailing constant 1.0 column, padded with zero
    rows to a fixed CAP), plus a per-row local slot index (0..127, the
    class's position within the group; pad rows get -1).
  - On device the scatter-add is a one-hot matmul on TensorE in fp8
    DoubleRow mode with a SINGLE 128-slot class tile:
        sums[slot, d] = sum_b onehot[b, slot] * embed[b, d]
    This is 8x fewer PE cycles than the data-parallel dense one-hot over
    all 1024 padded classes, and each core fully owns its classes so
    there is NO collective at all. The ones column makes the per-slot
    count fall out of the same matmuls (pad rows have an all-zero
    one-hot row, so they contribute neither sums nor counts).
  - epilogue per core: out = sums * (0.3/count) + 0.7*centroid for the
    core's <=128 slots; the host scatters slot rows back to class rows.

Schedule notes (v3): embed DMAs are spread over the sync/scalar/gpsimd
queues so descriptor issue (~0.6us each) pipelines and the 3 hardware
queues together reach the ~360 GB/s HBM ceiling; the slot-label DMA
goes out first on sync (it gates every one-hot build); all one-hot
builds stay on DVE (GpSimd's tensor_scalar is ~8x slower and cross-
engine writes to a shared tile stall both engines); the counts ride in
column 1024 of the embed tiles so no separate count matmuls exist.
"""

import numpy as np

import concourse.bacc as bacc
import concourse.mybir as mybir
import concourse.tile as tile
from concourse.bass_utils import run_bass_kernel_spmd

N_CORES = 8
C = 1000  # real classes
D = 1024  # embed dim
W = 1040  # embed + ones column at 1024 + zero pad (16B-aligned rows)
B = 32768  # total batch
P = 128
FACTOR = 0.3
# matmul column chunks (PSUM bank limit is 512 f32); the counts chunk
# (dims 896..1023 + the ones column at 1024 + pad) is computed FIRST per
# pair so the reciprocal can start as early as possible at the end
CHUNKS = [(896, 144), (0, 448), (448, 448)]

_F32 = mybir.dt.float32
_BF16 = mybir.dt.bfloat16
_FP8 = mybir.dt.float8e4

_CACHE: dict = {}


def _build(cap: int):
    kt = cap // P  # k-tiles per core
    kp = kt // 2  # k-pairs; DoubleRow consumes [128, 2, cols] per matmul

    nc = bacc.Bacc(
        "TRN2", target_bir_lowering=False, debug=False, num_devices=N_CORES
    )
    # emb8[p, k, :] = padded_rows[k*128 + p, :]; col D is constant 1.0
    emb8 = nc.dram_tensor("emb8", [P, kt, W], _FP8, kind="ExternalInput").ap()
    # yslot[p, k] = local slot (0..127) of padded row k*128+p, -1 for pads
    yslot = nc.dram_tensor("yslot", [P, kt], _F32, kind="ExternalInput").ap()
    cent = nc.dram_tensor("cent", [P, D], _BF16, kind="ExternalInput").ap()
    out = nc.dram_tensor("out", [P, D], _BF16, kind="ExternalOutput").ap()

    with tile.TileContext(nc) as tc:
        with (
            tc.tile_pool(name="const", bufs=1) as const_pool,
            tc.tile_pool(name="emb", bufs=kp) as emb_pool,
            tc.tile_pool(name="oh", bufs=kp) as oh_pool,
            tc.tile_pool(name="psum", bufs=1, space="PSUM") as psum_pool,
            tc.tile_pool(name="fin", bufs=1) as fin_pool,
        ):
            # first embed tile goes out before anything else (it gates the
            # first matmul); slot labels right behind (they gate the builds)
            emb_t0 = emb_pool.tile([P, 2, W], _FP8, name="emb0", tag="emb")
            nc.sync.dma_start(out=emb_t0[:], in_=emb8[:, 0:2, :])
            y_all = const_pool.tile([P, kt], _F32)
            nc.sync.dma_start(out=y_all[:], in_=yslot[:])

            # iota row replicated down all 128 partitions: iota[p, s] = s
            iota = const_pool.tile([P, P], _F32)
            nc.gpsimd.iota(
                iota[:],
                pattern=[[1, P]],
                base=0,
                channel_multiplier=0,
                allow_small_or_imprecise_dtypes=True,
            )

            psums = [
                psum_pool.tile([P, n], _F32, name=f"ps{q}")
                for q, (_, n) in enumerate(CHUNKS)
            ]

            emb_qs = [nc.sync, nc.scalar, nc.gpsimd]
            for j in range(kp):
                if j == 0:
                    emb_t = emb_t0
                else:
                    emb_t = emb_pool.tile(
                        [P, 2, W], _FP8, name=f"emb{j}", tag="emb"
                    )
                    emb_qs[j % 3].dma_start(
                        out=emb_t[:], in_=emb8[:, 2 * j : 2 * j + 2, :]
                    )
                oh_t = oh_pool.tile([P, 2, P], _FP8, name=f"oh{j}", tag="oh")
                for j2 in range(2):
                    k = 2 * j + j2
                    nc.vector.tensor_scalar(
                        oh_t[:, j2, :],
                        iota[:],
                        y_all[:, k : k + 1],
                        None,
                        mybir.AluOpType.is_equal,
                    )
                for q, (lo, n) in enumerate(CHUNKS):
                    nc.tensor.matmul(
                        psums[q][:],
                        lhsT=oh_t[:],
                        rhs=emb_t[:, :, lo : lo + n],
                        start=(j == 0),
                        stop=(j == kp - 1),
                        perf_mode=mybir.MatmulPerfMode.DoubleRow,
                    )

            # 0.7 * centroid, computed on ACT while the matmuls run
            # (issued after scalar's embed DMAs so they are not delayed)
            cent_sb = fin_pool.tile([P, D], _BF16, name="cent_sb")
            nc.scalar.dma_start(out=cent_sb[:], in_=cent[:])
            c07 = fin_pool.tile([P, D], _F32, name="c07")
            nc.scalar.mul(c07[:], cent_sb[:], 1.0 - FACTOR)

            # epilogue: recip = 0.3/count, out = sums*recip + 0.7*centroid
            # mults on ACT (per-partition scale), adds on DVE, pipelined
            recip = fin_pool.tile([P, 1], _F32, name="recip")
            nc.vector.reciprocal(recip[:], psums[0][:, 128:129])
            nc.vector.tensor_scalar(
                recip[:], recip[:], FACTOR, None, mybir.AluOpType.mult
            )
            # big chunks first; the tiny counts-chunk transfer (32KB) last
            out_qs = {0: nc.sync, 1: nc.sync, 2: nc.gpsimd}
            for q in (1, 2, 0):
                lo, n = CHUNKS[q]
                nd = 128 if q == 0 else n  # output dims in this chunk
                t1 = fin_pool.tile([P, nd], _F32, name=f"t1_{q}")
                nc.scalar.mul(t1[:], psums[q][:, 0:nd], recip[:, 0:1])
                out_sb = fin_pool.tile([P, nd], _BF16, name=f"o{q}")
                nc.vector.tensor_tensor(
                    out=out_sb[:],
                    in0=t1[:],
                    in1=c07[:, lo : lo + nd],
                    op=mybir.AluOpType.add,
                )
                out_qs[q].dma_start(out=out[:, lo : lo + nd], in_=out_sb[:])

    nc.compile()
    return nc


def get_nc(cap: int = 4096):
    if cap not in _CACHE:
        _CACHE[cap] = _build(cap)
    return _CACHE[cap]


def _partition_classes(counts: np.ndarray):
    """Split classes into N_CORES groups, <=128 classes each, minimizing
    the max total sample count. LPT greedy + swap refinement."""
    order = np.argsort(-counts, kind="stable")
    groups = [[] for _ in range(N_CORES)]
    sums = np.zeros(N_CORES, dtype=np.int64)
    for c in order:
        for b in np.argsort(sums, kind="stable"):
            if len(groups[b]) < P:
                groups[b].append(int(c))
                sums[b] += counts[c]
                break
    # swap refinement: push max bin down to the mean where possible
    target = int(np.ceil(counts.sum() / N_CORES))
    for _ in range(4000):
        hi = int(np.argmax(sums))
        if sums[hi] <= target:
            break
        lo = int(np.argmin(sums))
        diff = sums[hi] - sums[lo]
        best = None
        for ci in groups[hi]:
            for cj in groups[lo]:
                d = int(counts[ci]) - int(counts[cj])
                if 0 < d <= diff and (best is None or d > best[2]):
                    best = (ci, cj, d)
        if best is None:
            break
        ci, cj, _d = best
        groups[hi].remove(ci)
        groups[lo].remove(cj)
        groups[hi].append(cj)
        groups[lo].append(ci)
        sums[hi] -= _d
        sums[lo] += _d
    return groups, sums


def make_in_maps(embed: np.ndarray, y: np.ndarray, centroid: np.ndarray):
    fp8_np = mybir.dt.np(_FP8)
    bf16_np = mybir.dt.np(_BF16)
    embed8 = np.ascontiguousarray(embed, dtype=np.float32).astype(fp8_np)
    y = np.asarray(y).astype(np.int64)
    centroid = np.asarray(centroid, dtype=np.float32)
    counts = np.bincount(y, minlength=C)

    groups, sums = _partition_classes(counts)
    cap = max(4096, int(np.ceil(sums.max() / 256.0)) * 256)

    # class -> (core, slot) map
    core_of = np.full(C, -1, dtype=np.int64)
    slot_of = np.full(C, -1, dtype=np.int64)
    for i, g in enumerate(groups):
        for s, cls in enumerate(g):
            core_of[cls] = i
            slot_of[cls] = s

    kt = cap // P
    in_maps = []
    meta = []
    for i in range(N_CORES):
        rows = np.nonzero(core_of[y] == i)[0]
        n = rows.shape[0]
        emb_pad = np.zeros((cap, W), dtype=fp8_np)
        emb_pad[:n, :D] = embed8[rows]
        emb_pad[:, D] = 1.0  # counts column (pad rows are masked by onehot)
        # cols D+1..W-1 stay zero (row alignment pad)
        ys = np.full(cap, -1.0, dtype=np.float32)
        ys[:n] = slot_of[y[rows]].astype(np.float32)
        cent_i = np.zeros((P, D), dtype=np.float32)
        g = groups[i]
        cent_i[: len(g)] = centroid[g]
        in_maps.append(
            {
                # emb8[p, k, :] = emb_pad[k*128 + p, :]
                "emb8": np.ascontiguousarray(
                    emb_pad.reshape(kt, P, W).transpose(1, 0, 2)
                ),
                "yslot": np.ascontiguousarray(ys.reshape(kt, P).T),
                "cent": cent_i.astype(bf16_np),
            }
        )
        meta.append(g)
    return in_maps, meta, cap


def kernel(embed: np.ndarray, y: np.ndarray, centroid: np.ndarray) -> np.ndarray:
    in_maps, meta, cap = make_in_maps(embed, y, centroid)
    nc = get_nc(cap)
    res = run_bass_kernel_spmd(nc, in_maps, core_ids=list(range(N_CORES)))
    full = np.zeros((C, D), dtype=np.float32)
    for i in range(N_CORES):
        g = meta[i]
        full[g] = res.results[i]["out"][: len(g)].astype(np.float32)
    return full


# revision 13
# speedup vs baseline: 1.0290x; 1.0290x over previous
"""Trainium2 Bass kernel for the Centroid (segment_reduce) problem.

new_centroid = 0.3 * (segment_sum(embed, y) / counts) + 0.7 * centroid
  embed [32768, 1024] f32, y [32768] int (0..999), centroid [1000, 1024] f32

Strategy (8 NeuronCores, CLASS-parallel via host-side routing):
  - The host partitions the 1000 classes into 8 groups of <=128 classes,
    balanced by sample count (LPT + swap refinement; for the uniform
    label distribution every group lands at ~4096 of the 32768 samples).
  - Core i receives ONLY the embed rows whose label falls in its group
    (as fp8 e4m3 with a trailing constant 1.0 column, padded with zero
    rows to a fixed CAP), plus a per-row local slot index (0..127, the
    class's position within the group; pad rows get -1).
  - On device the scatter-add is a one-hot matmul on TensorE in fp8
    DoubleRow mode with a SINGLE 128-slot class tile:
        sums[slot, d] = sum_b onehot[b, slot] * embed[b, d]
    This is 8x fewer PE cycles than the data-parallel dense one-hot over
    all 1024 padded classes, and each core fully owns its classes so
    there is NO collective at all. The ones column makes the per-slot
    count fall out of the same matmuls (pad rows have an all-zero
    one-hot row, so they contribute neither sums nor counts).
  - epilogue per core: out = sums * (0.3/count) + 0.7*centroid for the
    core's <=128 slots; the host scatters slot rows back to class rows.

Schedule notes (v3): embed DMAs are spread over the sync/scalar/gpsimd
queues so descriptor issue (~0.6us each) pipelines and the 3 hardware
queues together reach the ~360 GB/s HBM ceiling; the slot-label DMA
goes out first on sync (it gates every one-hot build); all one-hot
builds stay on DVE (GpSimd's tensor_scalar is ~8x slower and cross-
engine writes to a shared tile stall both engines); the counts ride in
column 1024 of the embed tiles so no separate count matmuls exist.
"""

import numpy as np

import concourse.bacc as bacc
import concourse.mybir as mybir
import concourse.tile as tile
from concourse.bass_utils import run_bass_kernel_spmd

N_CORES = 8
C = 1000  # real classes
D = 1024  # embed dim
W = 1040  # embed + ones column at 1024 + zero pad (16B-aligned rows)
B = 32768  # total batch
P = 128
FACTOR = 0.3
# matmul column chunks (PSUM bank limit is 512 f32); the counts chunk
# (dims 896..1023 + the ones column at 1024 + pad) is computed FIRST per
# pair so the reciprocal can start as early as possible at the end
CHUNKS = [(896, 144), (0, 448), (448, 448)]

_F32 = mybir.dt.float32
_BF16 = mybir.dt.bfloat16
_FP8 = mybir.dt.float8e4

_CACHE: dict = {}


def _build(cap: int):
    kt = cap // P  # k-tiles per core
    kp = kt // 2  # k-pairs; DoubleRow consumes [128, 2, cols] per matmul

    nc = bacc.Bacc(
        "TRN2", target_bir_lowering=False, debug=False, num_devices=N_CORES
    )
    # emb8[p, k, :] = padded_rows[k*128 + p, :]; col D is constant 1.0
    emb8 = nc.dram_tensor("emb8", [P, kt, W], _FP8, kind="ExternalInput").ap()
    # yslot[p, k] = local slot (0..127) of padded row k*128+p, -1 for pads
    yslot = nc.dram_tensor("yslot", [P, kt], _F32, kind="ExternalInput").ap()
    cent = nc.dram_tensor("cent", [P, D], _BF16, kind="ExternalInput").ap()
    out = nc.dram_tensor("out", [P, D], _BF16, kind="ExternalOutput").ap()

    with tile.TileContext(nc) as tc:
        with (
            tc.tile_pool(name="const", bufs=1) as const_pool,
            tc.tile_pool(name="emb", bufs=kp) as emb_pool,
            tc.tile_pool(name="oh", bufs=kp) as oh_pool,
            tc.tile_pool(name="psum", bufs=1, space="PSUM") as psum_pool,
            tc.tile_pool(name="fin", bufs=1) as fin_pool,
        ):
            # first embed tile goes out before anything else (it gates the
            # first matmul); slot labels right behind (they gate the builds)
            emb_t0 = emb_pool.tile([P, 2, W], _FP8, name="emb0", tag="emb")
            nc.sync.dma_start(out=emb_t0[:], in_=emb8[:, 0:2, :])
            y_all = const_pool.tile([P, kt], _F32)
            nc.sync.dma_start(out=y_all[:], in_=yslot[:])

            # iota row replicated down all 128 partitions: iota[p, s] = s
            iota = const_pool.tile([P, P], _F32)
            nc.gpsimd.iota(
                iota[:],
                pattern=[[1, P]],
                base=0,
                channel_multiplier=0,
                allow_small_or_imprecise_dtypes=True,
            )

            psums = [
                psum_pool.tile([P, n], _F32, name=f"ps{q}")
                for q, (_, n) in enumerate(CHUNKS)
            ]

            emb_qs = [nc.sync, nc.scalar, nc.gpsimd]
            for j in range(kp):
                if j == 0:
                    emb_t = emb_t0
                else:
                    emb_t = emb_pool.tile(
                        [P, 2, W], _FP8, name=f"emb{j}", tag="emb"
                    )
                    emb_qs[j % 3].dma_start(
                        out=emb_t[:], in_=emb8[:, 2 * j : 2 * j + 2, :]
                    )
                oh_t = oh_pool.tile([P, 2, P], _FP8, name=f"oh{j}", tag="oh")
                for j2 in range(2):
                    k = 2 * j + j2
                    nc.vector.tensor_scalar(
                        oh_t[:, j2, :],
                        iota[:],
                        y_all[:, k : k + 1],
                        None,
                        mybir.AluOpType.is_equal,
                    )
                for q, (lo, n) in enumerate(CHUNKS):
                    nc.tensor.matmul(
                        psums[q][:],
                        lhsT=oh_t[:],
                        rhs=emb_t[:, :, lo : lo + n],
                        start=(j == 0),
                        stop=(j == kp - 1),
                        perf_mode=mybir.MatmulPerfMode.DoubleRow,
                    )

            # 0.7 * centroid, computed on ACT while the matmuls run
            # (issued after scalar's embed DMAs so they are not delayed)
            cent_sb = fin_pool.tile([P, D], _BF16, name="cent_sb")
            nc.scalar.dma_start(out=cent_sb[:], in_=cent[:])
            c07 = fin_pool.tile([P, D], _F32, name="c07")
            nc.scalar.mul(c07[:], cent_sb[:], 1.0 - FACTOR)

            # epilogue: recip = 0.3/count, out = sums*recip + 0.7*centroid
            # mults on ACT (per-partition scale), adds on DVE, pipelined
            recip = fin_pool.tile([P, 1], _F32, name="recip")
            nc.vector.reciprocal(recip[:], psums[0][:, 128:129])
            nc.vector.tensor_scalar(
                recip[:], recip[:], FACTOR, None, mybir.AluOpType.mult
            )
            # big chunks first; the tiny counts-chunk transfer (32KB) last
            out_qs = {0: nc.sync, 1: nc.sync, 2: nc.gpsimd}
            for q in (1, 2, 0):
                lo, n = CHUNKS[q]
                nd = 128 if q == 0 else n  # output dims in this chunk
                t1 = fin_pool.tile([P, nd], _F32, name=f"t1_{q}")
                nc.scalar.mul(t1[:], psums[q][:, 0:nd], recip[:, 0:1])
                out_sb = fin_pool.tile([P, nd], _BF16, name=f"o{q}")
                nc.vector.tensor_tensor(
                    out=out_sb[:],
                    in0=t1[:],
                    in1=c07[:, lo : lo + nd],
                    op=mybir.AluOpType.add,
                )
                out_qs[q].dma_start(out=out[:, lo : lo + nd], in_=out_sb[:])

    nc.compile()
    return nc


def get_nc(cap: int = 4096):
    if cap not in _CACHE:
        _CACHE[cap] = _build(cap)
    return _CACHE[cap]


def _refine(groups, sums, counts, target):
    """2-opt repair: swap classes between the max bin and any other bin
    whenever it strictly lowers max(pair); stop at max <= target."""
    for _ in range(6000):
        hi = int(np.argmax(sums))
        if sums[hi] <= target:
            return True
        best = None  # (new_pair_max, ci, cj, b, d)
        for b in range(N_CORES):
            if b == hi:
                continue
            for ci in groups[hi]:
                for cj in groups[b]:
                    d = int(counts[ci]) - int(counts[cj])
                    if d <= 0:
                        continue
                    m = max(sums[hi] - d, sums[b] + d)
                    if m < sums[hi] and (best is None or m < best[0]):
                        best = (m, ci, cj, b, d)
        if best is None:
            return False
        _m, ci, cj, b, d = best
        groups[hi].remove(ci)
        groups[b].remove(cj)
        groups[hi].append(cj)
        groups[b].append(ci)
        sums[hi] -= d
        sums[b] += d
    return bool(np.max(sums) <= target)


def _partition_classes(counts: np.ndarray):
    """Split classes into N_CORES groups, <=128 classes each, minimizing
    the max total sample count. LPT greedy + 2-opt repair, with a few
    deterministic randomized restarts to reach a perfect equipartition."""
    target = int(np.ceil(counts.sum() / N_CORES))
    order = np.argsort(-counts, kind="stable")
    best_groups, best_sums = None, None
    for seed in range(8):
        rng = np.random.default_rng(seed)
        groups = [[] for _ in range(N_CORES)]
        sums = np.zeros(N_CORES, dtype=np.int64)
        for c in order:
            cand = np.argsort(
                sums + (rng.integers(0, 2, N_CORES) if seed else 0),
                kind="stable",
            )
            for b in cand:
                if len(groups[b]) < P:
                    groups[b].append(int(c))
                    sums[b] += counts[c]
                    break
        ok = _refine(groups, sums, counts, target)
        if best_sums is None or sums.max() < best_sums.max():
            best_groups, best_sums = groups, sums
        if ok:
            break
    return best_groups, best_sums


def make_in_maps(embed: np.ndarray, y: np.ndarray, centroid: np.ndarray):
    fp8_np = mybir.dt.np(_FP8)
    bf16_np = mybir.dt.np(_BF16)
    embed8 = np.ascontiguousarray(embed, dtype=np.float32).astype(fp8_np)
    y = np.asarray(y).astype(np.int64)
    centroid = np.asarray(centroid, dtype=np.float32)
    counts = np.bincount(y, minlength=C)

    groups, sums = _partition_classes(counts)
    cap = max(4096, int(np.ceil(sums.max() / 256.0)) * 256)

    # class -> (core, slot) map
    core_of = np.full(C, -1, dtype=np.int64)
    slot_of = np.full(C, -1, dtype=np.int64)
    for i, g in enumerate(groups):
        for s, cls in enumerate(g):
            core_of[cls] = i
            slot_of[cls] = s

    kt = cap // P
    in_maps = []
    meta = []
    for i in range(N_CORES):
        rows = np.nonzero(core_of[y] == i)[0]
        n = rows.shape[0]
        emb_pad = np.zeros((cap, W), dtype=fp8_np)
        emb_pad[:n, :D] = embed8[rows]
        emb_pad[:, D] = 1.0  # counts column (pad rows are masked by onehot)
        # cols D+1..W-1 stay zero (row alignment pad)
        ys = np.full(cap, -1.0, dtype=np.float32)
        ys[:n] = slot_of[y[rows]].astype(np.float32)
        cent_i = np.zeros((P, D), dtype=np.float32)
        g = groups[i]
        cent_i[: len(g)] = centroid[g]
        in_maps.append(
            {
                # emb8[p, k, :] = emb_pad[k*128 + p, :]
                "emb8": np.ascontiguousarray(
                    emb_pad.reshape(kt, P, W).transpose(1, 0, 2)
                ),
                "yslot": np.ascontiguousarray(ys.reshape(kt, P).T),
                "cent": cent_i.astype(bf16_np),
            }
        )
        meta.append(g)
    return in_maps, meta, cap


def kernel(embed: np.ndarray, y: np.ndarray, centroid: np.ndarray) -> np.ndarray:
    in_maps, meta, cap = make_in_maps(embed, y, centroid)
    nc = get_nc(cap)
    res = run_bass_kernel_spmd(nc, in_maps, core_ids=list(range(N_CORES)))
    full = np.zeros((C, D), dtype=np.float32)
    for i in range(N_CORES):
        g = meta[i]
        full[g] = res.results[i]["out"][: len(g)].astype(np.float32)
    return full
